# revision 23
# baseline (speedup 1.0000x reference)
"""MinGRU Trainium2 kernel.

Reference computation (per batch b):
    c = depthwise_conv1d(x, conv_w, taps=5, pad=2)        # [D, L]
    h = h_w @ c                                           # [O, L]
    g = concat([-1000, +1000], g_w @ c)                   # [O, L]
    a = sigmoid(-g); v = sigmoid(g) * h
    out[l] = a[l] * out[l-1] + v[l]     (linear scan along L)

Strategy: pure data-parallel over B (8 batches -> 8 NeuronCores).
Per core, everything streams in l-chunks of 512 (processed in pairs so the
ScalarE activation table only switches twice per pair):
  - conv: 5 diagonal-matmuls on TensorE accumulating in PSUM (fp32r)
  - c PSUM->SBUF copies on ScalarE; h/g 1x1-conv matmuls on TensorE (fp32r)
  - a = sigmoid(-(g+bias)) on ScalarE (bias carries the +/-1000 polarized rows)
  - z = 1 - a on GpSimd, v = z*h on VectorE, scan via tensor_tensor_scan (DVE)
  - channel 0 output is exactly 0 (output buffers are pre-zeroed; never write)
  - channel 1 replicates the reference's f32 log-domain quantization
    out[1,l] = sign(h)*exp(fl(fl(K_l+ln|h|)-K_l)), K_l = 1000(l+1): h rows 0:2
    are packed into a [128,64] tile via SBUF->SBUF DMA, so the end-pass is a
    handful of full-width ops instead of a long [2,512] tail.
"""

import numpy as np

import concourse.bass as bass
import concourse.mybir as mybir
from concourse import bacc
from concourse.tile import TileContext
from concourse.bass_utils import run_bass_kernel_spmd

F32 = mybir.dt.float32
F32R = mybir.dt.float32r
U32 = mybir.dt.uint32
AF = mybir.ActivationFunctionType
OP = mybir.AluOpType

B, D, O, L = 8, 512, 512, 4096
P = 128
CH = 512                 # l-chunk width (one PSUM bank)
NCH = L // CH            # 8
NDT = D // P             # 4 d-tiles
NOT = O // P             # 4 o-tiles
NTAPS = 5
N_CORES = 8
PK = CH // 64            # 8 packed columns per chunk


def build_program():
    nc = bacc.Bacc()

    x = nc.declare_dram_parameter("x", [D, L], F32R, isOutput=False)
    hwT = nc.declare_dram_parameter("hwT", [D, O], F32R, isOutput=False)
    gwT = nc.declare_dram_parameter("gwT", [D, O], F32R, isOutput=False)
    cwdiag = nc.declare_dram_parameter("cwdiag", [D, NTAPS * P], F32R, isOutput=False)
    gbn = nc.declare_dram_parameter("gbn", [O, 1], F32, isOutput=False)
    kpack = nc.declare_dram_parameter("kpack", [P, 64], F32, isOutput=False)
    zpad = nc.declare_dram_parameter("zpad", [P, 2], F32R, isOutput=False)
    masks = nc.declare_dram_parameter("masks", [P, 2], U32, isOutput=False)
    out = nc.declare_dram_parameter("out", [O, L], F32, isOutput=True)

    with TileContext(nc) as tc:
        with (
            tc.tile_pool(name="weights", bufs=1) as wpool,
            tc.tile_pool(name="xin", bufs=6) as xpool,
            tc.tile_pool(name="csb", bufs=10) as cpool,
            tc.tile_pool(name="actout", bufs=4) as apool,
            tc.tile_pool(name="vtiles", bufs=4) as vpool,
            tc.tile_pool(name="outt", bufs=3) as opool,
            tc.tile_pool(name="cps", bufs=2, space="PSUM") as cps_pool,
            tc.tile_pool(name="hps", bufs=3, space="PSUM") as hps_pool,
            tc.tile_pool(name="gps", bufs=3, space="PSUM") as gps_pool,
        ):
            # all weights go through the GpSimd SWDGE queues so the Sync
            # HWDGE queue leads with the chunk-0 x tiles; conv weights first
            # (they gate the first matmuls), then h/g weights, then end-pass
            # constants.
            cw_sb, hwT_sb, gwT_sb, gbn_sb = [], [], [], []
            for dt in range(NDT):
                t = wpool.tile([P, NTAPS * P], F32R, tag=f"cw{dt}")
                nc.gpsimd.dma_start(out=t, in_=cwdiag[dt * P:(dt + 1) * P, :])
                cw_sb.append(t)
            for dt in range(NDT):
                t = wpool.tile([P, O], F32R, tag=f"hwT{dt}")
                nc.gpsimd.dma_start(out=t, in_=hwT[dt * P:(dt + 1) * P, :])
                hwT_sb.append(t)
                t = wpool.tile([P, O], F32R, tag=f"gwT{dt}")
                nc.gpsimd.dma_start(out=t, in_=gwT[dt * P:(dt + 1) * P, :])
                gwT_sb.append(t)
            for ot in range(NOT):
                t = wpool.tile([P, 1], F32, tag=f"gbn{ot}")
                nc.gpsimd.dma_start(out=t, in_=gbn[ot * P:(ot + 1) * P, :])
                gbn_sb.append(t)
            kpack_sb = wpool.tile([P, 64], F32, tag="kpack")
            nc.gpsimd.dma_start(out=kpack_sb, in_=kpack[:, :])
            masks_sb = wpool.tile([P, 2], U32, tag="masks")
            nc.gpsimd.dma_start(out=masks_sb, in_=masks[:, :])

            c_sb = [None] * NCH          # [chunk] -> list of 4 SBUF c tiles
            prev_out = [None] * NOT      # previous chunk's out tiles per o-tile
            hpack = wpool.tile([P, 64], F32, tag="hpack")

            def emit_conv(i):
                lo = i * CH
                tiles = []
                for dt in range(NDT):
                    # xt covers x columns [lo-2, lo+CH+2); halo columns that
                    # fall outside [0, L) are zero-filled via a tiny DMA from
                    # the zpad constant (keeps every producer fp32r-typed).
                    xt = xpool.tile([P, CH + 4], F32R, tag="xt")
                    if i == 0:
                        nc.sync.dma_start(out=xt[:, 0:2], in_=zpad[:, :])
                        nc.sync.dma_start(out=xt[:, 2:CH + 4],
                                          in_=x[dt * P:(dt + 1) * P, 0:CH + 2])
                    elif i == NCH - 1:
                        nc.sync.dma_start(out=xt[:, CH + 2:CH + 4], in_=zpad[:, :])
                        nc.sync.dma_start(out=xt[:, 0:CH + 2],
                                          in_=x[dt * P:(dt + 1) * P, lo - 2:lo + CH])
                    else:
                        nc.sync.dma_start(out=xt[:, :],
                                          in_=x[dt * P:(dt + 1) * P, lo - 2:lo + CH + 2])
                    cp = cps_pool.tile([P, CH], F32, tag="cps")
                    for k in range(NTAPS):
                        nc.tensor.matmul(
                            cp,
                            lhsT=cw_sb[dt][:, k * P:(k + 1) * P],
                            rhs=xt[:, k:k + CH],
                            start=(k == 0), stop=(k == NTAPS - 1),
                        )
                    ct = cpool.tile([P, CH], F32R, tag="ct")
                    nc.scalar.copy(ct, cp)
                    tiles.append(ct)
                c_sb[i] = tiles

            def emit_rest(i):
                lo = i * CH
                for ot in range(NOT):
                    hp = hps_pool.tile([P, CH], F32, tag="hps")
                    for dt in range(NDT):
                        nc.tensor.matmul(
                            hp,
                            lhsT=hwT_sb[dt][:, ot * P:(ot + 1) * P],
                            rhs=c_sb[i][dt],
                            start=(dt == 0), stop=(dt == NDT - 1),
                        )
                    gp = gps_pool.tile([P, CH], F32, tag="gps")
                    for dt in range(NDT):
                        nc.tensor.matmul(
                            gp,
                            lhsT=gwT_sb[dt][:, ot * P:(ot + 1) * P],
                            rhs=c_sb[i][dt],
                            start=(dt == 0), stop=(dt == NDT - 1),
                        )
                    # a = sigmoid(-(g + bias)) ; z = 1 - a ; v = z * h
                    at = apool.tile([P, CH], F32, tag="at")
                    nc.scalar.activation(at, gp, AF.Sigmoid, bias=gbn_sb[ot], scale=-1.0)
                    zt = vpool.tile([P, CH], F32, tag="zt")
                    nc.gpsimd.tensor_scalar(zt, at, -1.0, 1.0, OP.mult, OP.add)
                    vt = vpool.tile([P, CH], F32, tag="vt")
                    nc.vector.tensor_tensor(vt, zt, hp, OP.mult)
                    if ot == 0:
                        # stash h rows 0:2, packed across partitions:
                        # hpack[r*64+q, i*8+c] = h[r, i*512 + q*8 + c]
                        ht = wpool.tile([2, CH], F32, tag=f"hrow{i}")
                        nc.vector.tensor_copy(ht, hp[0:2, :])
                        nc.sync.dma_start(
                            out=hpack[:, i * PK:(i + 1) * PK],
                            in_=ht.rearrange("r (q c) -> r q c", c=PK))
                    ott = opool.tile([P, CH], F32, tag=f"out{ot}")
                    init = 0.0 if i == 0 else prev_out[ot][:, CH - 1:CH]
                    nc.vector.tensor_tensor_scan(ott, at, vt, init, OP.mult, OP.add)
                    if ot == 0:
                        # rows 0/1 are produced by the end-pass / pre-zeroing
                        nc.sync.dma_start(out=out[2:P, lo:lo + CH],
                                          in_=ott[2:P, :])
                    else:
                        nc.sync.dma_start(out=out[ot * P:(ot + 1) * P, lo:lo + CH],
                                          in_=ott)
                    prev_out[ot] = ott

            # chunk pairs, software-pipelined one pair ahead
            emit_conv(0)
            emit_conv(1)
            for p in range(1, NCH // 2):
                emit_conv(2 * p)
                emit_conv(2 * p + 1)
                emit_rest(2 * p - 2)
                emit_rest(2 * p - 1)
            emit_rest(NCH - 2)
            emit_rest(NCH - 1)

            # ---- end-pass: channel 1 on the packed [128, 64] tile ----
            # ch1 replicates the reference's f32 rounding:
            # out[1,l] = sign(h) * exp(fl(fl(K+ln|h|) - K)), K = 1000(l+1).
            absm = masks_sb[:, 0:1]
            sgnm = masks_sb[:, 1:2]
            t = wpool.tile([P, 64], F32, tag="ch1w")
            nc.vector.tensor_scalar(t.bitcast(U32), hpack.bitcast(U32),
                                    absm, None, OP.bitwise_and)
            nc.vector.tensor_scalar_max(t, t, 1e-6)
            nc.scalar.activation(t, t, AF.Ln)
            nc.vector.tensor_tensor(t, t, kpack_sb, OP.add)
            nc.vector.tensor_tensor(t, t, kpack_sb, OP.subtract)
            nc.scalar.activation(t, t, AF.Exp)
            res = wpool.tile([P, 64], F32, tag="ch1r")
            nc.vector.tensor_scalar(res.bitcast(U32), hpack.bitcast(U32),
                                    sgnm, None, OP.bitwise_and)
            nc.vector.tensor_tensor(res.bitcast(U32), res.bitcast(U32),
                                    t.bitcast(U32), OP.bitwise_or)
            for i in range(NCH):
                # row 1 only; row 0 of `out` stays pre-zeroed
                nc.sync.dma_start(
                    out=out[1:2, i * CH:(i + 1) * CH].rearrange(
                        "r (q c) -> r q c", c=PK),
                    in_=res[64:P, i * PK:(i + 1) * PK])

    nc.finalize()
    return nc


_PROGRAM = None


def _get_program():
    global _PROGRAM
    if _PROGRAM is None:
        _PROGRAM = build_program()
    return _PROGRAM


def prepare_in_maps(x, conv_w, h_w, g_w):
    x = np.ascontiguousarray(np.asarray(x), dtype=np.float32)
    conv_w = np.asarray(conv_w, dtype=np.float32)
    h_w = np.asarray(h_w, dtype=np.float32)
    g_w = np.asarray(g_w, dtype=np.float32)

    hwT = np.ascontiguousarray(h_w[:, :, 0].T)                    # [D, O]
    gw_pad = np.zeros((O, D), np.float32)
    gw_pad[2:, :] = g_w[:, :, 0]
    gwT = np.ascontiguousarray(gw_pad.T)                          # [D, O]

    # 5 diagonal matrices per d-tile, concatenated along free dim: [D, 5*128]
    cwdiag = np.zeros((D, NTAPS * P), np.float32)
    for dt in range(NDT):
        for k in range(NTAPS):
            blk = cwdiag[dt * P:(dt + 1) * P, k * P:(k + 1) * P]
            np.fill_diagonal(blk, conv_w[dt * P:(dt + 1) * P, 0, k])

    gbp = np.zeros((O, 1), np.float32)
    gbp[0, 0], gbp[1, 0] = -1000.0, 1000.0
    gbn = -gbp

    # K for the packed layout: kpack[r*64+q, i*8+c] = 1000*(i*512+q*8+c+1)
    q = np.arange(64)[:, None]
    i = np.repeat(np.arange(NCH), PK)[None, :]
    c = np.tile(np.arange(PK), NCH)[None, :]
    kp = 1000.0 * (i * CH + q * PK + c + 1.0)
    kpack = np.ascontiguousarray(
        np.concatenate([kp, kp], axis=0).astype(np.float32))      # [128, 64]

    zpad = np.zeros((P, 2), np.float32)
    masks = np.ascontiguousarray(np.broadcast_to(
        np.array([[0x7FFFFFFF, 0x80000000]], np.uint32), (P, 2)))
    return [
        {"x": x[b], "hwT": hwT, "gwT": gwT, "cwdiag": cwdiag,
         "gbn": gbn, "kpack": kpack, "zpad": zpad, "masks": masks}
        for b in range(B)
    ]


def kernel(x, conv_w, h_w, g_w):
    in_maps = prepare_in_maps(x, conv_w, h_w, g_w)
    nc = _get_program()
    res = run_bass_kernel_spmd(nc, in_maps, list(range(N_CORES))).results
    return np.stack([res[b]["out"] for b in range(B)], axis=0)


# revision 28
# speedup vs baseline: 1.1333x; 1.1333x over previous
"""MinGRU Trainium2 kernel.

Reference computation (per batch b):
    c = depthwise_conv1d(x, conv_w, taps=5, pad=2)        # [D, L]
    h = h_w @ c                                           # [O, L]
    g = concat([-1000, +1000], g_w @ c)                   # [O, L]
    a = sigmoid(-g); v = sigmoid(g) * h
    out[l] = a[l] * out[l-1] + v[l]     (linear scan along L)

Strategy: pure data-parallel over B (8 batches -> 8 NeuronCores).
Per core, everything streams in l-chunks of 512 (processed in pairs so the
ScalarE activation table only switches twice per pair):
  - conv: 5 diagonal-matmuls on TensorE accumulating in PSUM (fp32r)
  - c PSUM->SBUF copies on ScalarE; h/g 1x1-conv matmuls on TensorE (fp32r)
  - a = sigmoid(-(g+bias)) on ScalarE (bias carries the +/-1000 polarized rows)
  - z = 1 - a on GpSimd, v = z*h on VectorE, scan via tensor_tensor_scan (DVE)
  - channel 0 output is exactly 0 (output buffers are pre-zeroed; never write)
  - channel 1 replicates the reference's f32 log-domain quantization
    out[1,l] = sign(h)*exp(fl(fl(K_l+ln|h|)-K_l)), K_l = 1000(l+1): h rows 0:2
    are packed into a [128,64] tile via SBUF->SBUF DMA, so the end-pass is a
    handful of full-width ops instead of a long [2,512] tail.
"""

import numpy as np

import concourse.bass as bass
import concourse.mybir as mybir
from concourse import bacc
from concourse.tile import TileContext
from concourse.bass_utils import run_bass_kernel_spmd

F32 = mybir.dt.float32
F32R = mybir.dt.float32r
U32 = mybir.dt.uint32
AF = mybir.ActivationFunctionType
OP = mybir.AluOpType

B, D, O, L = 8, 512, 512, 4096
P = 128
CH = 512                 # l-chunk width (one PSUM bank)
NCH = L // CH            # 8
NDT = D // P             # 4 d-tiles
NOT = O // P             # 4 o-tiles
NTAPS = 5
N_CORES = 8
PK = CH // 64            # 8 packed columns per chunk


def build_program():
    nc = bacc.Bacc()

    x = nc.declare_dram_parameter("x", [D, L], F32R, isOutput=False)
    hwT = nc.declare_dram_parameter("hwT", [D, O], F32R, isOutput=False)
    gwT = nc.declare_dram_parameter("gwT", [D, O], F32R, isOutput=False)
    cwdiag = nc.declare_dram_parameter("cwdiag", [D, NTAPS * P], F32R, isOutput=False)
    gbn = nc.declare_dram_parameter("gbn", [O, 1], F32, isOutput=False)
    kpack = nc.declare_dram_parameter("kpack", [P, 64], F32, isOutput=False)
    zpad = nc.declare_dram_parameter("zpad", [P, 2], F32R, isOutput=False)
    masks = nc.declare_dram_parameter("masks", [P, 2], U32, isOutput=False)
    out = nc.declare_dram_parameter("out", [O, L], F32, isOutput=True)

    with TileContext(nc) as tc:
        with (
            tc.tile_pool(name="weights", bufs=1) as wpool,
            tc.tile_pool(name="xin", bufs=6) as xpool,
            tc.tile_pool(name="csb", bufs=10) as cpool,
            tc.tile_pool(name="actout", bufs=4) as apool,
            tc.tile_pool(name="vtiles", bufs=4) as vpool,
            tc.tile_pool(name="outt", bufs=3) as opool,
            tc.tile_pool(name="cps", bufs=2, space="PSUM") as cps_pool,
            tc.tile_pool(name="hps", bufs=3, space="PSUM") as hps_pool,
            tc.tile_pool(name="gps", bufs=3, space="PSUM") as gps_pool,
        ):
            # h/g weight matrices go through the GpSimd SWDGE queues, in
            # parallel with the Sync HWDGE queue which leads with the chunk-0
            # x tiles (emitted inside emit_conv(0)); the conv weights and
            # small constants follow them on Sync (emitted below, after
            # emit_conv(0)).
            # conv weights + small constants go out on the Scalar engine's
            # DMA queues, h/g weights on the GpSimd queues; the Sync queue is
            # left free to lead with the chunk-0 x tiles.
            cw_sb, hwT_sb, gwT_sb, gbn_sb = [], [], [], []
            for dt in range(NDT):
                t = wpool.tile([P, NTAPS * P], F32R, tag=f"cw{dt}")
                nc.scalar.dma_start(out=t, in_=cwdiag[dt * P:(dt + 1) * P, :])
                cw_sb.append(t)
            for ot in range(NOT):
                t = wpool.tile([P, 1], F32, tag=f"gbn{ot}")
                nc.scalar.dma_start(out=t, in_=gbn[ot * P:(ot + 1) * P, :])
                gbn_sb.append(t)
            kpack_sb = wpool.tile([P, 64], F32, tag="kpack")
            nc.scalar.dma_start(out=kpack_sb, in_=kpack[:, :])
            masks_sb = wpool.tile([P, 2], U32, tag="masks")
            nc.scalar.dma_start(out=masks_sb, in_=masks[:, :])
            for dt in range(NDT):
                t = wpool.tile([P, O], F32R, tag=f"hwT{dt}")
                nc.gpsimd.dma_start(out=t, in_=hwT[dt * P:(dt + 1) * P, :])
                hwT_sb.append(t)
                t = wpool.tile([P, O], F32R, tag=f"gwT{dt}")
                nc.gpsimd.dma_start(out=t, in_=gwT[dt * P:(dt + 1) * P, :])
                gwT_sb.append(t)

            c_sb = [None] * NCH          # [chunk] -> list of 4 SBUF c tiles
            prev_out = [None] * NOT      # previous chunk's out tiles per o-tile
            hpack = wpool.tile([P, 64], F32, tag="hpack")

            def emit_conv(i):
                lo = i * CH
                tiles = []
                for dt in range(NDT):
                    # xt covers x columns [lo-2, lo+CH+2); halo columns that
                    # fall outside [0, L) are zero-filled via a tiny DMA from
                    # the zpad constant (keeps every producer fp32r-typed).
                    xt = xpool.tile([P, CH + 4], F32R, tag="xt")
                    if i == 0:
                        nc.sync.dma_start(out=xt[:, 0:2], in_=zpad[:, :])
                        nc.sync.dma_start(out=xt[:, 2:CH + 4],
                                          in_=x[dt * P:(dt + 1) * P, 0:CH + 2])
                    elif i == NCH - 1:
                        nc.sync.dma_start(out=xt[:, CH + 2:CH + 4], in_=zpad[:, :])
                        nc.sync.dma_start(out=xt[:, 0:CH + 2],
                                          in_=x[dt * P:(dt + 1) * P, lo - 2:lo + CH])
                    else:
                        nc.sync.dma_start(out=xt[:, :],
                                          in_=x[dt * P:(dt + 1) * P, lo - 2:lo + CH + 2])
                    cp = cps_pool.tile([P, CH], F32, tag="cps")
                    for k in range(NTAPS):
                        nc.tensor.matmul(
                            cp,
                            lhsT=cw_sb[dt][:, k * P:(k + 1) * P],
                            rhs=xt[:, k:k + CH],
                            start=(k == 0), stop=(k == NTAPS - 1),
                        )
                    ct = cpool.tile([P, CH], F32R, tag="ct")
                    nc.scalar.copy(ct, cp)
                    tiles.append(ct)
                c_sb[i] = tiles

            def emit_rest(i):
                lo = i * CH
                for ot in range(NOT):
                    hp = hps_pool.tile([P, CH], F32, tag="hps")
                    for dt in range(NDT):
                        nc.tensor.matmul(
                            hp,
                            lhsT=hwT_sb[dt][:, ot * P:(ot + 1) * P],
                            rhs=c_sb[i][dt],
                            start=(dt == 0), stop=(dt == NDT - 1),
                        )
                    gp = gps_pool.tile([P, CH], F32, tag="gps")
                    for dt in range(NDT):
                        nc.tensor.matmul(
                            gp,
                            lhsT=gwT_sb[dt][:, ot * P:(ot + 1) * P],
                            rhs=c_sb[i][dt],
                            start=(dt == 0), stop=(dt == NDT - 1),
                        )
                    # a = sigmoid(-(g + bias)) ; z = 1 - a ; v = z * h
                    at = apool.tile([P, CH], F32, tag="at")
                    nc.scalar.activation(at, gp, AF.Sigmoid, bias=gbn_sb[ot], scale=-1.0)
                    zt = vpool.tile([P, CH], F32, tag="zt")
                    nc.gpsimd.tensor_scalar(zt, at, -1.0, 1.0, OP.mult, OP.add)
                    vt = vpool.tile([P, CH], F32, tag="vt")
                    nc.vector.tensor_tensor(vt, zt, hp, OP.mult)
                    if ot == 0:
                        # stash h rows 0:2, packed across partitions:
                        # hpack[r*64+q, i*8+c] = h[r, i*512 + q*8 + c]
                        ht = wpool.tile([2, CH], F32, tag=f"hrow{i}")
                        nc.vector.tensor_copy(ht, hp[0:2, :])
                        nc.sync.dma_start(
                            out=hpack[:, i * PK:(i + 1) * PK],
                            in_=ht.rearrange("r (q c) -> r q c", c=PK))
                    ott = opool.tile([P, CH], F32, tag=f"out{ot}")
                    init = 0.0 if i == 0 else prev_out[ot][:, CH - 1:CH]
                    nc.vector.tensor_tensor_scan(ott, at, vt, init, OP.mult, OP.add)
                    if ot == 0:
                        # rows 0/1 are produced by the end-pass / pre-zeroing
                        nc.sync.dma_start(out=out[2:P, lo:lo + CH],
                                          in_=ott[2:P, :])
                    else:
                        nc.sync.dma_start(out=out[ot * P:(ot + 1) * P, lo:lo + CH],
                                          in_=ott)
                    prev_out[ot] = ott

            # chunk pairs, software-pipelined one pair ahead
            emit_conv(0)
            emit_conv(1)
            for p in range(1, NCH // 2):
                emit_conv(2 * p)
                emit_conv(2 * p + 1)
                emit_rest(2 * p - 2)
                emit_rest(2 * p - 1)
            emit_rest(NCH - 2)
            emit_rest(NCH - 1)

            # ---- end-pass: channel 1 on the packed [128, 64] tile ----
            # ch1 replicates the reference's f32 rounding:
            # out[1,l] = sign(h) * exp(fl(fl(K+ln|h|) - K)), K = 1000(l+1).
            absm = masks_sb[:, 0:1]
            sgnm = masks_sb[:, 1:2]
            t = wpool.tile([P, 64], F32, tag="ch1w")
            nc.vector.tensor_scalar(t.bitcast(U32), hpack.bitcast(U32),
                                    absm, None, OP.bitwise_and)
            nc.vector.tensor_scalar_max(t, t, 1e-6)
            nc.scalar.activation(t, t, AF.Ln)
            nc.vector.tensor_tensor(t, t, kpack_sb, OP.add)
            nc.vector.tensor_tensor(t, t, kpack_sb, OP.subtract)
            nc.scalar.activation(t, t, AF.Exp)
            res = wpool.tile([P, 64], F32, tag="ch1r")
            nc.vector.tensor_scalar(res.bitcast(U32), hpack.bitcast(U32),
                                    sgnm, None, OP.bitwise_and)
            nc.vector.tensor_tensor(res.bitcast(U32), res.bitcast(U32),
                                    t.bitcast(U32), OP.bitwise_or)
            for i in range(NCH):
                # row 1 only; row 0 of `out` stays pre-zeroed
                nc.sync.dma_start(
                    out=out[1:2, i * CH:(i + 1) * CH].rearrange(
                        "r (q c) -> r q c", c=PK),
                    in_=res[64:P, i * PK:(i + 1) * PK])

    nc.finalize()
    return nc


_PROGRAM = None


def _get_program():
    global _PROGRAM
    if _PROGRAM is None:
        _PROGRAM = build_program()
    return _PROGRAM


def prepare_in_maps(x, conv_w, h_w, g_w):
    x = np.ascontiguousarray(np.asarray(x), dtype=np.float32)
    conv_w = np.asarray(conv_w, dtype=np.float32)
    h_w = np.asarray(h_w, dtype=np.float32)
    g_w = np.asarray(g_w, dtype=np.float32)

    hwT = np.ascontiguousarray(h_w[:, :, 0].T)                    # [D, O]
    gw_pad = np.zeros((O, D), np.float32)
    gw_pad[2:, :] = g_w[:, :, 0]
    gwT = np.ascontiguousarray(gw_pad.T)                          # [D, O]

    # 5 diagonal matrices per d-tile, concatenated along free dim: [D, 5*128]
    cwdiag = np.zeros((D, NTAPS * P), np.float32)
    for dt in range(NDT):
        for k in range(NTAPS):
            blk = cwdiag[dt * P:(dt + 1) * P, k * P:(k + 1) * P]
            np.fill_diagonal(blk, conv_w[dt * P:(dt + 1) * P, 0, k])

    gbp = np.zeros((O, 1), np.float32)
    gbp[0, 0], gbp[1, 0] = -1000.0, 1000.0
    gbn = -gbp

    # K for the packed layout: kpack[r*64+q, i*8+c] = 1000*(i*512+q*8+c+1)
    q = np.arange(64)[:, None]
    i = np.repeat(np.arange(NCH), PK)[None, :]
    c = np.tile(np.arange(PK), NCH)[None, :]
    kp = 1000.0 * (i * CH + q * PK + c + 1.0)
    kpack = np.ascontiguousarray(
        np.concatenate([kp, kp], axis=0).astype(np.float32))      # [128, 64]

    zpad = np.zeros((P, 2), np.float32)
    masks = np.ascontiguousarray(np.broadcast_to(
        np.array([[0x7FFFFFFF, 0x80000000]], np.uint32), (P, 2)))
    return [
        {"x": x[b], "hwT": hwT, "gwT": gwT, "cwdiag": cwdiag,
         "gbn": gbn, "kpack": kpack, "zpad": zpad, "masks": masks}
        for b in range(B)
    ]


def kernel(x, conv_w, h_w, g_w):
    in_maps = prepare_in_maps(x, conv_w, h_w, g_w)
    nc = _get_program()
    res = run_bass_kernel_spmd(nc, in_maps, list(range(N_CORES))).results
    return np.stack([res[b]["out"] for b in range(B)], axis=0)


# revision 32
# speedup vs baseline: 1.1471x; 1.0122x over previous
"""MinGRU Trainium2 kernel.

Reference computation (per batch b):
    c = depthwise_conv1d(x, conv_w, taps=5, pad=2)        # [D, L]
    h = h_w @ c                                           # [O, L]
    g = concat([-1000, +1000], g_w @ c)                   # [O, L]
    a = sigmoid(-g); v = sigmoid(g) * h
    out[l] = a[l] * out[l-1] + v[l]     (linear scan along L)

Strategy: pure data-parallel over B (8 batches -> 8 NeuronCores).
Per core, everything streams in l-chunks of 512 (processed in pairs so the
ScalarE activation table only switches twice per pair):
  - conv: 5 diagonal-matmuls on TensorE accumulating in PSUM (fp32r)
  - c PSUM->SBUF copies on ScalarE; h/g 1x1-conv matmuls on TensorE (fp32r)
  - a = sigmoid(-(g+bias)) on ScalarE (bias carries the +/-1000 polarized rows)
  - z = 1 - a on GpSimd, v = z*h on VectorE, scan via tensor_tensor_scan (DVE)
  - channel 0 output is exactly 0 (output buffers are pre-zeroed; never write)
  - channel 1 replicates the reference's f32 log-domain quantization
    out[1,l] = sign(h)*exp(fl(fl(K_l+ln|h|)-K_l)), K_l = 1000(l+1): h rows 0:2
    are packed into a [128,64] tile via SBUF->SBUF DMA, so the end-pass is a
    handful of full-width ops instead of a long [2,512] tail.
"""

import numpy as np

import concourse.bass as bass
import concourse.mybir as mybir
from concourse import bacc
from concourse.tile import TileContext
from concourse.bass_utils import run_bass_kernel_spmd

F32 = mybir.dt.float32
F32R = mybir.dt.float32r
U32 = mybir.dt.uint32
AF = mybir.ActivationFunctionType
OP = mybir.AluOpType

B, D, O, L = 8, 512, 512, 4096
P = 128
CH = 512                 # l-chunk width (one PSUM bank)
NCH = L // CH            # 8
NDT = D // P             # 4 d-tiles
NOT = O // P             # 4 o-tiles
NTAPS = 5
N_CORES = 8
PK = CH // 64            # 8 packed columns per chunk


def build_program():
    nc = bacc.Bacc()

    x = nc.declare_dram_parameter("x", [D, L], F32R, isOutput=False)
    hwT = nc.declare_dram_parameter("hwT", [D, O], F32R, isOutput=False)
    gwT = nc.declare_dram_parameter("gwT", [D, O], F32R, isOutput=False)
    cwdiag = nc.declare_dram_parameter("cwdiag", [D, NTAPS * P], F32R, isOutput=False)
    gbn = nc.declare_dram_parameter("gbn", [O, 1], F32, isOutput=False)
    kpack = nc.declare_dram_parameter("kpack", [P, 64], F32, isOutput=False)
    zpad = nc.declare_dram_parameter("zpad", [P, 2], F32R, isOutput=False)
    masks = nc.declare_dram_parameter("masks", [P, 2], U32, isOutput=False)
    out = nc.declare_dram_parameter("out", [O, L], F32, isOutput=True)

    with TileContext(nc) as tc:
        with (
            tc.tile_pool(name="weights", bufs=1) as wpool,
            tc.tile_pool(name="xin", bufs=6) as xpool,
            tc.tile_pool(name="csb", bufs=10) as cpool,
            tc.tile_pool(name="actout", bufs=4) as apool,
            tc.tile_pool(name="vtiles", bufs=4) as vpool,
            tc.tile_pool(name="outt", bufs=3) as opool,
            tc.tile_pool(name="cps", bufs=2, space="PSUM") as cps_pool,
            tc.tile_pool(name="hps", bufs=3, space="PSUM") as hps_pool,
            tc.tile_pool(name="gps", bufs=3, space="PSUM") as gps_pool,
        ):
            # h/g weight matrices go through the GpSimd SWDGE queues, in
            # parallel with the Sync HWDGE queue which leads with the chunk-0
            # x tiles (emitted inside emit_conv(0)); the conv weights and
            # small constants follow them on Sync (emitted below, after
            # emit_conv(0)).
            # conv weights + small constants go out on the Scalar engine's
            # DMA queues, h/g weights on the GpSimd queues; the Sync queue is
            # left free to lead with the chunk-0 x tiles.
            cw_sb, hwT_sb, gwT_sb, gbn_sb = [], [], [], []
            for dt in range(NDT):
                t = wpool.tile([P, NTAPS * P], F32R, tag=f"cw{dt}")
                nc.scalar.dma_start(out=t, in_=cwdiag[dt * P:(dt + 1) * P, :])
                cw_sb.append(t)
            for ot in range(NOT):
                t = wpool.tile([P, 1], F32, tag=f"gbn{ot}")
                nc.scalar.dma_start(out=t, in_=gbn[ot * P:(ot + 1) * P, :])
                gbn_sb.append(t)
            kpack_sb = wpool.tile([P, 64], F32, tag="kpack")
            nc.scalar.dma_start(out=kpack_sb, in_=kpack[:, :])
            masks_sb = wpool.tile([P, 2], U32, tag="masks")
            nc.scalar.dma_start(out=masks_sb, in_=masks[:, :])
            for dt in range(NDT):
                t = wpool.tile([P, O], F32R, tag=f"hwT{dt}")
                nc.gpsimd.dma_start(out=t, in_=hwT[dt * P:(dt + 1) * P, :])
                hwT_sb.append(t)
                t = wpool.tile([P, O], F32R, tag=f"gwT{dt}")
                nc.gpsimd.dma_start(out=t, in_=gwT[dt * P:(dt + 1) * P, :])
                gwT_sb.append(t)

            c_sb = [None] * NCH          # [chunk] -> list of 4 SBUF c tiles
            prev_out = [None] * NOT      # previous chunk's out tiles per o-tile
            hrow = [None] * NCH          # [chunk] -> [2, CH] copy of h rows 0:2
            hpack = wpool.tile([P, 64], F32, tag="hpack")

            def emit_conv(i):
                lo = i * CH
                tiles = []
                for dt in range(NDT):
                    # xt covers x columns [lo-2, lo+CH+2); halo columns that
                    # fall outside [0, L) are zero-filled via a tiny DMA from
                    # the zpad constant (keeps every producer fp32r-typed).
                    xt = xpool.tile([P, CH + 4], F32R, tag="xt")
                    if i == 0:
                        nc.sync.dma_start(out=xt[:, 0:2], in_=zpad[:, :])
                        nc.sync.dma_start(out=xt[:, 2:CH + 4],
                                          in_=x[dt * P:(dt + 1) * P, 0:CH + 2])
                    elif i == NCH - 1:
                        nc.sync.dma_start(out=xt[:, CH + 2:CH + 4], in_=zpad[:, :])
                        nc.sync.dma_start(out=xt[:, 0:CH + 2],
                                          in_=x[dt * P:(dt + 1) * P, lo - 2:lo + CH])
                    else:
                        nc.sync.dma_start(out=xt[:, :],
                                          in_=x[dt * P:(dt + 1) * P, lo - 2:lo + CH + 2])
                    cp = cps_pool.tile([P, CH], F32, tag="cps")
                    for k in range(NTAPS):
                        nc.tensor.matmul(
                            cp,
                            lhsT=cw_sb[dt][:, k * P:(k + 1) * P],
                            rhs=xt[:, k:k + CH],
                            start=(k == 0), stop=(k == NTAPS - 1),
                        )
                    ct = cpool.tile([P, CH], F32R, tag="ct")
                    nc.scalar.copy(ct, cp)
                    tiles.append(ct)
                c_sb[i] = tiles

            def emit_rest(i):
                lo = i * CH
                for ot in range(NOT):
                    hp = hps_pool.tile([P, CH], F32, tag="hps")
                    for dt in range(NDT):
                        nc.tensor.matmul(
                            hp,
                            lhsT=hwT_sb[dt][:, ot * P:(ot + 1) * P],
                            rhs=c_sb[i][dt],
                            start=(dt == 0), stop=(dt == NDT - 1),
                        )
                    gp = gps_pool.tile([P, CH], F32, tag="gps")
                    for dt in range(NDT):
                        nc.tensor.matmul(
                            gp,
                            lhsT=gwT_sb[dt][:, ot * P:(ot + 1) * P],
                            rhs=c_sb[i][dt],
                            start=(dt == 0), stop=(dt == NDT - 1),
                        )
                    # a = sigmoid(-(g + bias)) ; z = 1 - a ; v = z * h
                    at = apool.tile([P, CH], F32, tag="at")
                    nc.scalar.activation(at, gp, AF.Sigmoid, bias=gbn_sb[ot], scale=-1.0)
                    zt = vpool.tile([P, CH], F32, tag="zt")
                    nc.gpsimd.tensor_scalar(zt, at, -1.0, 1.0, OP.mult, OP.add)
                    vt = vpool.tile([P, CH], F32, tag="vt")
                    nc.vector.tensor_tensor(vt, zt, hp, OP.mult)
                    if ot == 0 and hrow[i] is None:
                        # stash h rows 0:2, packed across partitions:
                        # hpack[r*64+q, i*8+c] = h[r, i*512 + q*8 + c]
                        ht = wpool.tile([2, CH], F32, tag=f"hrow{i}")
                        nc.vector.tensor_copy(ht, hp[0:2, :])
                        nc.sync.dma_start(
                            out=hpack[:, i * PK:(i + 1) * PK],
                            in_=ht.rearrange("r (q c) -> r q c", c=PK))
                        hrow[i] = ht
                    ott = opool.tile([P, CH], F32, tag=f"out{ot}")
                    init = 0.0 if i == 0 else prev_out[ot][:, CH - 1:CH]
                    nc.vector.tensor_tensor_scan(ott, at, vt, init,
                                                 OP.mult, OP.add)
                    if ot == 0:
                        # rows 0/1 are produced by the end-pass / pre-zeroing
                        nc.sync.dma_start(out=out[2:P, lo:lo + CH],
                                          in_=ott[2:P, :])
                    else:
                        nc.sync.dma_start(out=out[ot * P:(ot + 1) * P, lo:lo + CH],
                                          in_=ott)
                    prev_out[ot] = ott

            def emit_hrow_early(i):
                # h rows 0:2 for chunk i via a tiny 2-row matmul so the
                # end-pass doesn't have to wait for the full h of the last
                # chunks.
                cpx = cps_pool.tile([P, CH], F32, tag="cps", name="cpx")
                for dt in range(NDT):
                    nc.tensor.matmul(
                        cpx[0:2, :],
                        lhsT=hwT_sb[dt][:, 0:2],
                        rhs=c_sb[i][dt],
                        start=(dt == 0), stop=(dt == NDT - 1),
                    )
                ht = wpool.tile([2, CH], F32, tag=f"hrow{i}", name=f"hrowE{i}")
                nc.vector.tensor_copy(ht, cpx[0:2, :])
                nc.sync.dma_start(
                    out=hpack[:, i * PK:(i + 1) * PK],
                    in_=ht.rearrange("r (q c) -> r q c", c=PK))
                hrow[i] = ht

            def emit_endpass():
                # ---- channel 1 on the packed [128, 64] tile ----
                # replicates the reference's f32 rounding:
                # out[1,l] = sign(h)*exp(fl(fl(K+ln|h|) - K)), K = 1000(l+1).
                absm = masks_sb[:, 0:1]
                sgnm = masks_sb[:, 1:2]
                t = wpool.tile([P, 64], F32, tag="ch1w", name="ch1w")
                nc.vector.tensor_scalar(t.bitcast(U32), hpack.bitcast(U32),
                                        absm, None, OP.bitwise_and)
                nc.vector.tensor_scalar_max(t, t, 1e-6)
                nc.scalar.activation(t, t, AF.Ln)
                nc.vector.tensor_tensor(t, t, kpack_sb, OP.add)
                nc.vector.tensor_tensor(t, t, kpack_sb, OP.subtract)
                nc.scalar.activation(t, t, AF.Exp)
                res = wpool.tile([P, 64], F32, tag="ch1r", name="ch1r")
                nc.vector.tensor_scalar(res.bitcast(U32), hpack.bitcast(U32),
                                        sgnm, None, OP.bitwise_and)
                nc.vector.tensor_tensor(res.bitcast(U32), res.bitcast(U32),
                                        t.bitcast(U32), OP.bitwise_or)
                for i in range(NCH):
                    # row 1 only; row 0 of `out` stays pre-zeroed
                    nc.sync.dma_start(
                        out=out[1:2, i * CH:(i + 1) * CH].rearrange(
                            "r (q c) -> r q c", c=PK),
                        in_=res[64:P, i * PK:(i + 1) * PK])

            # chunk pairs, software-pipelined one pair ahead
            emit_conv(0)
            emit_conv(1)
            for p in range(1, NCH // 2):
                emit_conv(2 * p)
                emit_conv(2 * p + 1)
                if p == NCH // 2 - 1:
                    emit_hrow_early(NCH - 2)
                    emit_hrow_early(NCH - 1)
                emit_rest(2 * p - 2)
                emit_rest(2 * p - 1)
                if p == NCH // 2 - 1:
                    emit_endpass()
            emit_rest(NCH - 2)
            emit_rest(NCH - 1)

    nc.finalize()
    return nc


_PROGRAM = None


def _get_program():
    global _PROGRAM
    if _PROGRAM is None:
        _PROGRAM = build_program()
    return _PROGRAM


def prepare_in_maps(x, conv_w, h_w, g_w):
    x = np.ascontiguousarray(np.asarray(x), dtype=np.float32)
    conv_w = np.asarray(conv_w, dtype=np.float32)
    h_w = np.asarray(h_w, dtype=np.float32)
    g_w = np.asarray(g_w, dtype=np.float32)

    hwT = np.ascontiguousarray(h_w[:, :, 0].T)                    # [D, O]
    gw_pad = np.zeros((O, D), np.float32)
    gw_pad[2:, :] = g_w[:, :, 0]
    gwT = np.ascontiguousarray(gw_pad.T)                          # [D, O]

    # 5 diagonal matrices per d-tile, concatenated along free dim: [D, 5*128]
    cwdiag = np.zeros((D, NTAPS * P), np.float32)
    for dt in range(NDT):
        for k in range(NTAPS):
            blk = cwdiag[dt * P:(dt + 1) * P, k * P:(k + 1) * P]
            np.fill_diagonal(blk, conv_w[dt * P:(dt + 1) * P, 0, k])

    gbp = np.zeros((O, 1), np.float32)
    gbp[0, 0], gbp[1, 0] = -1000.0, 1000.0
    gbn = -gbp

    # K for the packed layout: kpack[r*64+q, i*8+c] = 1000*(i*512+q*8+c+1)
    q = np.arange(64)[:, None]
    i = np.repeat(np.arange(NCH), PK)[None, :]
    c = np.tile(np.arange(PK), NCH)[None, :]
    kp = 1000.0 * (i * CH + q * PK + c + 1.0)
    kpack = np.ascontiguousarray(
        np.concatenate([kp, kp], axis=0).astype(np.float32))      # [128, 64]

    zpad = np.zeros((P, 2), np.float32)
    masks = np.ascontiguousarray(np.broadcast_to(
        np.array([[0x7FFFFFFF, 0x80000000]], np.uint32), (P, 2)))
    return [
        {"x": x[b], "hwT": hwT, "gwT": gwT, "cwdiag": cwdiag,
         "gbn": gbn, "kpack": kpack, "zpad": zpad, "masks": masks}
        for b in range(B)
    ]


def kernel(x, conv_w, h_w, g_w):
    in_maps = prepare_in_maps(x, conv_w, h_w, g_w)
    nc = _get_program()
    res = run_bass_kernel_spmd(nc, in_maps, list(range(N_CORES))).results
    return np.stack([res[b]["out"] for b in range(B)], axis=0)
